# revision 7
# baseline (speedup 1.0000x reference)
"""Euclidean distance block (retrieval kNN) on 8 TRN2 NeuronCores.

dist[b, s, p] = sqrt(sum_c (x1[b, c, p] - x2[b, s, c, p])^2)   p = spatial (h*w)
out[b] = dist[b].reshape(S * h * w)

Sharding: data-parallel over batch B=32 -> 4 batches per core, no comms.

Per-core kernel layout: SBUF partitions carry (support_pair, channel) = 2*64 =
128; the free axis carries spatial. A big tile covers 8 supports as
[128, 4, 1764] (one fully-contiguous 3.6 MB DMA). Compute chain per tile:
  DVE subtract (x1 broadcast over the support axis)
  ACT Square
  PE matmul against a [128, 2] ones-block mask -> per-support sums over C
    into a [25, 441] PSUM tile per spatial quarter (partition = support)
  ACT Sqrt PSUM -> SBUF, one contiguous 176 KB store per batch.
"""

import numpy as np

B, S, C, H, W = 32, 25, 64, 42, 42
HW = H * W            # 1764
NCORES = 8
BL = B // NCORES      # 4 batches per core
NSO = 4               # support pairs per big tile (8 supports)
NBIG = 3              # big tiles per batch (24 supports), then 1 leftover
NQ = 4                # spatial quarters
QW = HW // NQ         # 441
NPAIR = 13            # 12 support pairs + 1 leftover single

_cache = {}


def _build_nc():
    import concourse.bacc as bacc
    import concourse.mybir as mybir
    from concourse.tile import TileContext
    from concourse.bass import MemorySpace

    f32 = mybir.dt.float32
    Square = mybir.ActivationFunctionType.Square
    Sqrt = mybir.ActivationFunctionType.Sqrt
    sub = mybir.AluOpType.subtract

    nc = bacc.Bacc()
    x1 = nc.declare_dram_parameter("x1", [BL, C, HW], f32, isOutput=False)
    x2 = nc.declare_dram_parameter("x2", [BL, S, C, HW], f32, isOutput=False)
    mk = nc.declare_dram_parameter("mask", [NPAIR, 128, S], f32, isOutput=False)
    out = nc.declare_dram_parameter("out", [BL, S * HW], f32, isOutput=True)

    with TileContext(nc) as tc:
        with (
            tc.tile_pool(name="x2p", bufs=2) as x2p,
            tc.tile_pool(name="sqp", bufs=2) as sqp,
            tc.tile_pool(name="x1p", bufs=2) as x1p,
            tc.tile_pool(name="outp", bufs=2) as outp,
            tc.tile_pool(name="cst", bufs=1) as cst,
            tc.tile_pool(name="ps", bufs=2, space=MemorySpace.PSUM) as psp,
        ):
            mt = cst.tile([128, NPAIR, S], f32)
            nc.sync.dma_start(mt[:], mk.rearrange("g k m -> k g m"))

            for b in range(BL):
                x1d = x1p.tile([128, HW], f32, tag="x1d")
                nc.sync.dma_start(x1d[0:64, :], x1[b])
                nc.sync.dma_start(x1d[64:128, :], x1[b])
                x1b = x1d[:, None, :].to_broadcast([128, NSO, HW])

                pst = [
                    psp.tile([S, QW], f32, name=f"ps{q}", tag=f"ps{q}")
                    for q in range(NQ)
                ]

                for i in range(NBIG):
                    x2t = x2p.tile([128, NSO, HW], f32, tag="x2t")
                    src = x2[b, 8 * i : 8 * i + 8].rearrange(
                        "(so si) c p -> (si c) so p", si=2
                    )
                    nc.sync.dma_start(x2t[:], src)
                    # in-place: x2t becomes diff
                    nc.vector.tensor_tensor(x2t[:], x2t[:], x1b, sub)
                    sq = sqp.tile([128, NSO, HW], f32, tag="sq")
                    nc.scalar.activation(sq[:], x2t[:], Square)
                    for so in range(NSO):
                        j = NSO * i + so
                        for q in range(NQ):
                            nc.tensor.matmul(
                                pst[q][:, :],
                                mt[:, j, :],
                                sq[:, so, q * QW : (q + 1) * QW],
                                start=(j == 0),
                                stop=False,
                            )

                # leftover support 24 on 64 partitions
                x2l = x2p.tile([64, HW], f32, tag="x2l")
                nc.sync.dma_start(x2l[:], x2[b, S - 1])
                nc.vector.tensor_tensor(x2l[:], x2l[:], x1d[0:64, :], sub)
                sql = sqp.tile([64, HW], f32, tag="sql")
                nc.scalar.activation(sql[:], x2l[:], Square)
                for q in range(NQ):
                    nc.tensor.matmul(
                        pst[q][:, :],
                        mt[0:64, NPAIR - 1, :],
                        sql[:, q * QW : (q + 1) * QW],
                        start=False,
                        stop=True,
                    )

                ot = outp.tile([S, HW], f32, tag="ot")
                for q in range(NQ):
                    nc.scalar.activation(ot[:, q * QW : (q + 1) * QW], pst[q][:], Sqrt)
                nc.sync.dma_start(out[b].rearrange("(s p) -> s p", s=S), ot[:])

    nc.finalize()
    return nc


def get_nc():
    if "nc" not in _cache:
        _cache["nc"] = _build_nc()
    return _cache["nc"]


def make_mask() -> np.ndarray:
    # mask[j, k, m] = 1 iff partition k of pair-tile j feeds output support m.
    # Pair j < 12 covers supports (2j, 2j+1): k < 64 -> 2j, k >= 64 -> 2j+1.
    # Pair 12 is the leftover single support 24 on partitions 0..63.
    mask = np.zeros((NPAIR, 128, S), dtype=np.float32)
    for j in range(NPAIR - 1):
        mask[j, 0:64, 2 * j] = 1.0
        mask[j, 64:128, 2 * j + 1] = 1.0
    mask[NPAIR - 1, 0:64, S - 1] = 1.0
    return mask


def make_in_maps(x1: np.ndarray, x2: np.ndarray) -> list[dict]:
    x1 = np.ascontiguousarray(np.asarray(x1, dtype=np.float32)).reshape(B, C, HW)
    x2 = np.ascontiguousarray(np.asarray(x2, dtype=np.float32)).reshape(B, S, C, HW)
    mask = make_mask()
    maps = []
    for i in range(NCORES):
        sl = slice(i * BL, (i + 1) * BL)
        maps.append({"x1": x1[sl], "x2": x2[sl], "mask": mask})
    return maps


def gather_out(results: list[dict]) -> np.ndarray:
    return np.concatenate([np.asarray(r["out"]) for r in results], axis=0).astype(
        np.float32, copy=False
    )


def kernel(x1, x2) -> np.ndarray:
    from concourse.bass_utils import run_bass_kernel_spmd

    nc = get_nc()
    in_maps = make_in_maps(x1, x2)
    res = run_bass_kernel_spmd(nc, in_maps, list(range(NCORES)))
    return gather_out(res.results)


# revision 8
# speedup vs baseline: 1.0578x; 1.0578x over previous
"""Euclidean distance block (retrieval kNN) on 8 TRN2 NeuronCores.

dist[b, s, p] = sqrt(sum_c (x1[b, c, p] - x2[b, s, c, p])^2)   p = spatial (h*w)
out[b] = dist[b].reshape(S * h * w)

Sharding: data-parallel over batch B=32 -> 4 batches per core, no comms.

Per-core kernel layout: SBUF partitions carry (support_pair, channel) = 2*64 =
128; the free axis carries spatial. A big tile covers 8 supports as
[128, 4, 1764] (one fully-contiguous 3.6 MB DMA). Compute chain per tile:
  DVE subtract (x1 broadcast over the support axis)
  ACT Square
  PE matmul against a [128, 2] ones-block mask -> per-support sums over C
    into a [25, 441] PSUM tile per spatial quarter (partition = support)
  ACT Sqrt PSUM -> SBUF, one contiguous 176 KB store per batch.
"""

import numpy as np

B, S, C, H, W = 32, 25, 64, 42, 42
HW = H * W            # 1764
NCORES = 8
BL = B // NCORES      # 4 batches per core
NSO = 4               # support pairs per big tile (8 supports)
NBIG = 3              # big tiles per batch (24 supports), then 1 leftover
NQ = 4                # spatial quarters
QW = HW // NQ         # 441
NPAIR = 13            # 12 support pairs + 1 leftover single

_cache = {}


def _build_nc():
    import concourse.bacc as bacc
    import concourse.mybir as mybir
    from concourse.tile import TileContext
    from concourse.bass import MemorySpace

    f32 = mybir.dt.float32
    bf16 = mybir.dt.bfloat16
    Square = mybir.ActivationFunctionType.Square
    Sqrt = mybir.ActivationFunctionType.Sqrt
    sub = mybir.AluOpType.subtract

    nc = bacc.Bacc()
    x1 = nc.declare_dram_parameter("x1", [BL, C, HW], f32, isOutput=False)
    x2 = nc.declare_dram_parameter("x2", [BL, S, C, HW], f32, isOutput=False)
    mk = nc.declare_dram_parameter("mask", [NPAIR, 128, S], bf16, isOutput=False)
    out = nc.declare_dram_parameter("out", [BL, S * HW], f32, isOutput=True)

    with TileContext(nc) as tc:
        with (
            tc.tile_pool(name="x2p", bufs=2) as x2p,
            tc.tile_pool(name="sqp", bufs=2) as sqp,
            tc.tile_pool(name="x1p", bufs=2) as x1p,
            tc.tile_pool(name="outp", bufs=2) as outp,
            tc.tile_pool(name="cst", bufs=1) as cst,
            tc.tile_pool(name="ps", bufs=2, space=MemorySpace.PSUM) as psp,
        ):
            mt = cst.tile([128, NPAIR, S], bf16)
            nc.sync.dma_start(mt[:], mk.rearrange("g k m -> k g m"))

            for b in range(BL):
                x1d = x1p.tile([128, HW], f32, tag="x1d")
                nc.sync.dma_start(x1d[0:64, :], x1[b])
                nc.sync.dma_start(x1d[64:128, :], x1[b])
                x1b = x1d[:, None, :].to_broadcast([128, NSO, HW])

                pst = [
                    psp.tile([S, QW], f32, name=f"ps{q}", tag=f"ps{q}")
                    for q in range(NQ)
                ]

                for i in range(NBIG):
                    x2t = x2p.tile([128, NSO, HW], f32, tag="x2t")
                    src = x2[b, 8 * i : 8 * i + 8].rearrange(
                        "(so si) c p -> (si c) so p", si=2
                    )
                    nc.sync.dma_start(x2t[:], src)
                    # in-place: x2t becomes diff
                    nc.vector.tensor_tensor(x2t[:], x2t[:], x1b, sub)
                    sq = sqp.tile([128, NSO, HW], bf16, tag="sq")
                    nc.scalar.activation(sq[:], x2t[:], Square)
                    for so in range(NSO):
                        j = NSO * i + so
                        for q in range(NQ):
                            nc.tensor.matmul(
                                pst[q][:, :],
                                mt[:, j, :],
                                sq[:, so, q * QW : (q + 1) * QW],
                                start=(j == 0),
                                stop=False,
                            )

                # leftover support 24 on 64 partitions
                x2l = x2p.tile([64, HW], f32, tag="x2l")
                nc.sync.dma_start(x2l[:], x2[b, S - 1])
                nc.vector.tensor_tensor(x2l[:], x2l[:], x1d[0:64, :], sub)
                sql = sqp.tile([64, HW], bf16, tag="sql")
                nc.scalar.activation(sql[:], x2l[:], Square)
                for q in range(NQ):
                    nc.tensor.matmul(
                        pst[q][:, :],
                        mt[0:64, NPAIR - 1, :],
                        sql[:, q * QW : (q + 1) * QW],
                        start=False,
                        stop=True,
                    )

                ot = outp.tile([S, HW], f32, tag="ot")
                for q in range(NQ):
                    nc.scalar.activation(ot[:, q * QW : (q + 1) * QW], pst[q][:], Sqrt)
                nc.sync.dma_start(out[b].rearrange("(s p) -> s p", s=S), ot[:])

    nc.finalize()
    return nc


def get_nc():
    if "nc" not in _cache:
        _cache["nc"] = _build_nc()
    return _cache["nc"]


def make_mask() -> np.ndarray:
    # mask[j, k, m] = 1 iff partition k of pair-tile j feeds output support m.
    # Pair j < 12 covers supports (2j, 2j+1): k < 64 -> 2j, k >= 64 -> 2j+1.
    # Pair 12 is the leftover single support 24 on partitions 0..63.
    import ml_dtypes

    mask = np.zeros((NPAIR, 128, S), dtype=ml_dtypes.bfloat16)
    for j in range(NPAIR - 1):
        mask[j, 0:64, 2 * j] = 1.0
        mask[j, 64:128, 2 * j + 1] = 1.0
    mask[NPAIR - 1, 0:64, S - 1] = 1.0
    return mask


def make_in_maps(x1: np.ndarray, x2: np.ndarray) -> list[dict]:
    x1 = np.ascontiguousarray(np.asarray(x1, dtype=np.float32)).reshape(B, C, HW)
    x2 = np.ascontiguousarray(np.asarray(x2, dtype=np.float32)).reshape(B, S, C, HW)
    mask = make_mask()
    maps = []
    for i in range(NCORES):
        sl = slice(i * BL, (i + 1) * BL)
        maps.append({"x1": x1[sl], "x2": x2[sl], "mask": mask})
    return maps


def gather_out(results: list[dict]) -> np.ndarray:
    return np.concatenate([np.asarray(r["out"]) for r in results], axis=0).astype(
        np.float32, copy=False
    )


def kernel(x1, x2) -> np.ndarray:
    from concourse.bass_utils import run_bass_kernel_spmd

    nc = get_nc()
    in_maps = make_in_maps(x1, x2)
    res = run_bass_kernel_spmd(nc, in_maps, list(range(NCORES)))
    return gather_out(res.results)


# revision 9
# speedup vs baseline: 1.1183x; 1.0572x over previous
"""Euclidean distance block (retrieval kNN) on 8 TRN2 NeuronCores.

dist[b, s, p] = sqrt(sum_c (x1[b, c, p] - x2[b, s, c, p])^2)   p = spatial (h*w)
out[b] = dist[b].reshape(S * h * w)

Sharding: data-parallel over batch B=32 -> 4 batches per core, no comms.

Per-core kernel layout: SBUF partitions carry (support_pair, channel) = 2*64 =
128; the free axis carries spatial. A big tile covers 8 supports as
[128, 4, 1764] (one fully-contiguous 3.6 MB DMA). Compute chain per tile:
  DVE subtract (x1 broadcast over the support axis)
  ACT Square
  PE matmul against a [128, 2] ones-block mask -> per-support sums over C
    into a [25, 441] PSUM tile per spatial quarter (partition = support)
  ACT Sqrt PSUM -> SBUF, one contiguous 176 KB store per batch.
"""

import numpy as np

B, S, C, H, W = 32, 25, 64, 42, 42
HW = H * W            # 1764
NCORES = 8
BL = B // NCORES      # 4 batches per core
NSO = 4               # support pairs per big tile (8 supports)
NBIG = 3              # big tiles per batch (24 supports), then 1 leftover
NQ = 4                # spatial quarters
QW = HW // NQ         # 441
NPAIR = 13            # 12 support pairs + 1 leftover single

_cache = {}


def _build_nc():
    import concourse.bacc as bacc
    import concourse.mybir as mybir
    from concourse.tile import TileContext
    from concourse.bass import MemorySpace

    f32 = mybir.dt.float32
    bf16 = mybir.dt.bfloat16
    Square = mybir.ActivationFunctionType.Square
    Sqrt = mybir.ActivationFunctionType.Sqrt
    sub = mybir.AluOpType.subtract

    nc = bacc.Bacc()
    x1 = nc.declare_dram_parameter("x1", [BL, C, HW], f32, isOutput=False)
    x2 = nc.declare_dram_parameter("x2", [BL, S, C, HW], f32, isOutput=False)
    mk = nc.declare_dram_parameter("mask", [NPAIR, 128, S], bf16, isOutput=False)
    out = nc.declare_dram_parameter("out", [BL, S * HW], f32, isOutput=True)

    with TileContext(nc) as tc:
        with (
            tc.tile_pool(name="x2p", bufs=3) as x2p,
            tc.tile_pool(name="sqp", bufs=2) as sqp,
            tc.tile_pool(name="x1p", bufs=2) as x1p,
            tc.tile_pool(name="outp", bufs=2) as outp,
            tc.tile_pool(name="cst", bufs=1) as cst,
            tc.tile_pool(name="ps", bufs=2, space=MemorySpace.PSUM) as psp,
        ):
            mt = cst.tile([128, NPAIR, S], bf16)
            nc.sync.dma_start(mt[:], mk.rearrange("g k m -> k g m"))

            for b in range(BL):
                x1d = x1p.tile([128, HW], f32, tag="x1d")
                nc.sync.dma_start(x1d[0:64, :], x1[b])
                nc.sync.dma_start(x1d[64:128, :], x1[b])
                x1b = x1d[:, None, :].to_broadcast([128, NSO, HW])

                pst = [
                    psp.tile([S, QW], f32, name=f"ps{q}", tag=f"ps{q}")
                    for q in range(NQ)
                ]

                for i in range(NBIG):
                    x2t = x2p.tile([128, NSO, HW], f32, tag="x2t")
                    src = x2[b, 8 * i : 8 * i + 8].rearrange(
                        "(so si) c p -> (si c) so p", si=2
                    )
                    nc.sync.dma_start(x2t[:], src)
                    # in-place: x2t becomes diff
                    nc.vector.tensor_tensor(x2t[:], x2t[:], x1b, sub)
                    sq = sqp.tile([128, NSO, HW], bf16, tag="sq")
                    nc.scalar.activation(sq[:], x2t[:], Square)
                    for so in range(NSO):
                        j = NSO * i + so
                        for q in range(NQ):
                            nc.tensor.matmul(
                                pst[q][:, :],
                                mt[:, j, :],
                                sq[:, so, q * QW : (q + 1) * QW],
                                start=(j == 0),
                                stop=False,
                            )

                # leftover support 24 on 64 partitions
                x2l = x2p.tile([64, HW], f32, tag="x2l")
                nc.sync.dma_start(x2l[:], x2[b, S - 1])
                nc.vector.tensor_tensor(x2l[:], x2l[:], x1d[0:64, :], sub)
                sql = sqp.tile([64, HW], bf16, tag="sql")
                nc.scalar.activation(sql[:], x2l[:], Square)
                for q in range(NQ):
                    nc.tensor.matmul(
                        pst[q][:, :],
                        mt[0:64, NPAIR - 1, :],
                        sql[:, q * QW : (q + 1) * QW],
                        start=False,
                        stop=True,
                    )

                ot = outp.tile([S, HW], f32, tag="ot")
                for q in range(NQ):
                    nc.scalar.activation(ot[:, q * QW : (q + 1) * QW], pst[q][:], Sqrt)
                nc.sync.dma_start(out[b].rearrange("(s p) -> s p", s=S), ot[:])

    nc.finalize()
    return nc


def get_nc():
    if "nc" not in _cache:
        _cache["nc"] = _build_nc()
    return _cache["nc"]


def make_mask() -> np.ndarray:
    # mask[j, k, m] = 1 iff partition k of pair-tile j feeds output support m.
    # Pair j < 12 covers supports (2j, 2j+1): k < 64 -> 2j, k >= 64 -> 2j+1.
    # Pair 12 is the leftover single support 24 on partitions 0..63.
    import ml_dtypes

    mask = np.zeros((NPAIR, 128, S), dtype=ml_dtypes.bfloat16)
    for j in range(NPAIR - 1):
        mask[j, 0:64, 2 * j] = 1.0
        mask[j, 64:128, 2 * j + 1] = 1.0
    mask[NPAIR - 1, 0:64, S - 1] = 1.0
    return mask


def make_in_maps(x1: np.ndarray, x2: np.ndarray) -> list[dict]:
    x1 = np.ascontiguousarray(np.asarray(x1, dtype=np.float32)).reshape(B, C, HW)
    x2 = np.ascontiguousarray(np.asarray(x2, dtype=np.float32)).reshape(B, S, C, HW)
    mask = make_mask()
    maps = []
    for i in range(NCORES):
        sl = slice(i * BL, (i + 1) * BL)
        maps.append({"x1": x1[sl], "x2": x2[sl], "mask": mask})
    return maps


def gather_out(results: list[dict]) -> np.ndarray:
    return np.concatenate([np.asarray(r["out"]) for r in results], axis=0).astype(
        np.float32, copy=False
    )


def kernel(x1, x2) -> np.ndarray:
    from concourse.bass_utils import run_bass_kernel_spmd

    nc = get_nc()
    in_maps = make_in_maps(x1, x2)
    res = run_bass_kernel_spmd(nc, in_maps, list(range(NCORES)))
    return gather_out(res.results)


# revision 10
# speedup vs baseline: 1.1258x; 1.0067x over previous
"""Euclidean distance block (retrieval kNN) on 8 TRN2 NeuronCores.

dist[b, s, p] = sqrt(sum_c (x1[b, c, p] - x2[b, s, c, p])^2)   p = spatial (h*w)
out[b] = dist[b].reshape(S * h * w)

Sharding: data-parallel over batch B=32 -> 4 batches per core, no comms.

Per-core kernel layout: SBUF partitions carry (support_pair, channel) = 2*64 =
128; the free axis carries spatial. A big tile covers 8 supports as
[128, 4, 1764] (one fully-contiguous 3.6 MB DMA). Compute chain per tile:
  DVE subtract (x1 broadcast over the support axis)
  ACT Square
  PE matmul against a [128, 2] ones-block mask -> per-support sums over C
    into a [25, 441] PSUM tile per spatial quarter (partition = support)
  ACT Sqrt PSUM -> SBUF, one contiguous 176 KB store per batch.
"""

import numpy as np

B, S, C, H, W = 32, 25, 64, 42, 42
HW = H * W            # 1764
NCORES = 8
BL = B // NCORES      # 4 batches per core
NSO = 4               # support pairs per big tile (8 supports)
NBIG = 3              # big tiles per batch (24 supports), then 1 leftover
NQ = 4                # spatial quarters
QW = HW // NQ         # 441
NPAIR = 13            # 12 support pairs + 1 leftover single

_cache = {}


def _build_nc():
    import concourse.bacc as bacc
    import concourse.mybir as mybir
    from concourse.tile import TileContext
    from concourse.bass import MemorySpace

    f32 = mybir.dt.float32
    bf16 = mybir.dt.bfloat16
    Square = mybir.ActivationFunctionType.Square
    Sqrt = mybir.ActivationFunctionType.Sqrt
    sub = mybir.AluOpType.subtract

    nc = bacc.Bacc()
    x1 = nc.declare_dram_parameter("x1", [BL, C, HW], f32, isOutput=False)
    x2 = nc.declare_dram_parameter("x2", [BL, S, C, HW], f32, isOutput=False)
    mk = nc.declare_dram_parameter("mask", [NPAIR, 128, S], bf16, isOutput=False)
    out = nc.declare_dram_parameter("out", [BL, S * HW], f32, isOutput=True)

    with TileContext(nc) as tc:
        with (
            tc.tile_pool(name="x2p", bufs=3) as x2p,
            tc.tile_pool(name="sqp", bufs=2) as sqp,
            tc.tile_pool(name="x1p", bufs=2) as x1p,
            tc.tile_pool(name="outp", bufs=2) as outp,
            tc.tile_pool(name="cst", bufs=1) as cst,
            tc.tile_pool(name="ps", bufs=2, space=MemorySpace.PSUM) as psp,
        ):
            mt = cst.tile([128, NPAIR, S], bf16)
            nc.sync.dma_start(mt[:], mk.rearrange("g k m -> k g m"))

            for b in range(BL):
                x1d = x1p.tile([128, HW], f32, tag="x1d")
                nc.sync.dma_start(x1d[0:64, :], x1[b])
                nc.sync.dma_start(x1d[64:128, :], x1[b])

                # leftover support 24: DMA early so it streams with big tiles
                x2l = x2p.tile([64, HW], f32, tag="x2l")
                nc.sync.dma_start(x2l[:], x2[b, S - 1])

                pst = [
                    psp.tile([S, QW], f32, name=f"ps{q}", tag=f"ps{q}")
                    for q in range(NQ)
                ]

                for i in range(NBIG):
                    x2t = x2p.tile([128, NSO, HW], f32, tag="x2t")
                    src = x2[b, 8 * i : 8 * i + 8].rearrange(
                        "(so si) c p -> (si c) so p", si=2
                    )
                    nc.sync.dma_start(x2t[:], src)
                    sq = sqp.tile([128, NSO, HW], bf16, tag="sq")
                    for so in range(NSO):
                        # in-place: x2t slice becomes diff
                        nc.vector.tensor_tensor(
                            x2t[:, so, :], x2t[:, so, :], x1d[:], sub
                        )
                        nc.scalar.activation(sq[:, so, :], x2t[:, so, :], Square)
                        j = NSO * i + so
                        for q in range(NQ):
                            nc.tensor.matmul(
                                pst[q][:, :],
                                mt[:, j, :],
                                sq[:, so, q * QW : (q + 1) * QW],
                                start=(j == 0),
                                stop=False,
                            )

                nc.vector.tensor_tensor(x2l[:], x2l[:], x1d[0:64, :], sub)
                sql = sqp.tile([64, HW], bf16, tag="sql")
                nc.scalar.activation(sql[:], x2l[:], Square)
                for q in range(NQ):
                    nc.tensor.matmul(
                        pst[q][:, :],
                        mt[0:64, NPAIR - 1, :],
                        sql[:, q * QW : (q + 1) * QW],
                        start=False,
                        stop=True,
                    )

                ot = outp.tile([S, HW], f32, tag="ot")
                for q in range(NQ):
                    nc.scalar.activation(ot[:, q * QW : (q + 1) * QW], pst[q][:], Sqrt)
                nc.sync.dma_start(out[b].rearrange("(s p) -> s p", s=S), ot[:])

    nc.finalize()
    return nc


def get_nc():
    if "nc" not in _cache:
        _cache["nc"] = _build_nc()
    return _cache["nc"]


def make_mask() -> np.ndarray:
    # mask[j, k, m] = 1 iff partition k of pair-tile j feeds output support m.
    # Pair j < 12 covers supports (2j, 2j+1): k < 64 -> 2j, k >= 64 -> 2j+1.
    # Pair 12 is the leftover single support 24 on partitions 0..63.
    import ml_dtypes

    mask = np.zeros((NPAIR, 128, S), dtype=ml_dtypes.bfloat16)
    for j in range(NPAIR - 1):
        mask[j, 0:64, 2 * j] = 1.0
        mask[j, 64:128, 2 * j + 1] = 1.0
    mask[NPAIR - 1, 0:64, S - 1] = 1.0
    return mask


def make_in_maps(x1: np.ndarray, x2: np.ndarray) -> list[dict]:
    x1 = np.ascontiguousarray(np.asarray(x1, dtype=np.float32)).reshape(B, C, HW)
    x2 = np.ascontiguousarray(np.asarray(x2, dtype=np.float32)).reshape(B, S, C, HW)
    mask = make_mask()
    maps = []
    for i in range(NCORES):
        sl = slice(i * BL, (i + 1) * BL)
        maps.append({"x1": x1[sl], "x2": x2[sl], "mask": mask})
    return maps


def gather_out(results: list[dict]) -> np.ndarray:
    return np.concatenate([np.asarray(r["out"]) for r in results], axis=0).astype(
        np.float32, copy=False
    )


def kernel(x1, x2) -> np.ndarray:
    from concourse.bass_utils import run_bass_kernel_spmd

    nc = get_nc()
    in_maps = make_in_maps(x1, x2)
    res = run_bass_kernel_spmd(nc, in_maps, list(range(NCORES)))
    return gather_out(res.results)


# revision 11
# speedup vs baseline: 1.2844x; 1.1409x over previous
"""Euclidean distance block (retrieval kNN) on 8 TRN2 NeuronCores.

dist[b, s, p] = sqrt(sum_c (x1[b, c, p] - x2[b, s, c, p])^2)   p = spatial (h*w)
out[b] = dist[b].reshape(S * h * w)

Sharding: data-parallel over batch B=32 -> 4 batches per core, no comms.

Per-core kernel layout: SBUF partitions carry (support_pair, channel) = 2*64 =
128; the free axis carries spatial. A big tile covers 8 supports as
[128, 4, 1764] (one fully-contiguous 3.6 MB DMA). Compute chain per tile:
  DVE subtract (x1 broadcast over the support axis)
  ACT Square
  PE matmul against a [128, 2] ones-block mask -> per-support sums over C
    into a [25, 441] PSUM tile per spatial quarter (partition = support)
  ACT Sqrt PSUM -> SBUF, one contiguous 176 KB store per batch.
"""

import numpy as np

B, S, C, H, W = 32, 25, 64, 42, 42
HW = H * W            # 1764
NCORES = 8
BL = B // NCORES      # 4 batches per core
NSO = 4               # support pairs per big tile (8 supports)
NBIG = 3              # big tiles per batch (24 supports), then 1 leftover
NQ = 4                # spatial quarters
QW = HW // NQ         # 441
NPAIR = 13            # 12 support pairs + 1 leftover single

_cache = {}


def _build_nc():
    import concourse.bacc as bacc
    import concourse.mybir as mybir
    from concourse.tile import TileContext
    from concourse.bass import MemorySpace

    f32 = mybir.dt.float32
    bf16 = mybir.dt.bfloat16
    Square = mybir.ActivationFunctionType.Square
    Sqrt = mybir.ActivationFunctionType.Sqrt
    sub = mybir.AluOpType.subtract

    nc = bacc.Bacc()
    x1 = nc.declare_dram_parameter("x1", [BL, C, HW], f32, isOutput=False)
    x2 = nc.declare_dram_parameter("x2", [BL, S, C, HW], f32, isOutput=False)
    mk = nc.declare_dram_parameter("mask", [NPAIR, 128, S], bf16, isOutput=False)
    out = nc.declare_dram_parameter("out", [BL, S * HW], f32, isOutput=True)

    with TileContext(nc) as tc:
        with (
            tc.tile_pool(name="x2p", bufs=3) as x2p,
            tc.tile_pool(name="sqp", bufs=2) as sqp,
            tc.tile_pool(name="x1p", bufs=2) as x1p,
            tc.tile_pool(name="outp", bufs=2) as outp,
            tc.tile_pool(name="cst", bufs=1) as cst,
            tc.tile_pool(name="ps", bufs=2, space=MemorySpace.PSUM) as psp,
        ):
            mt = cst.tile([128, NPAIR, S], bf16)
            nc.sync.dma_start(mt[:], mk.rearrange("g k m -> k g m"))

            for b in range(BL):
                x1d = x1p.tile([128, HW], f32, tag="x1d")
                nc.sync.dma_start(x1d[0:64, :], x1[b])
                nc.sync.dma_start(x1d[64:128, :], x1[b])

                # leftover support 24: DMA early so it streams with big tiles
                x2l = x2p.tile([64, HW], f32, tag="x2l")
                nc.sync.dma_start(x2l[:], x2[b, S - 1])

                pst = [
                    psp.tile([S, QW], f32, name=f"ps{q}", tag=f"ps{q}")
                    for q in range(NQ)
                ]

                for i in range(NBIG):
                    x2t = x2p.tile([128, NSO, HW], f32, tag="x2t")
                    src = x2[b, 8 * i : 8 * i + 8].rearrange(
                        "(so si) c p -> (si c) so p", si=2
                    )
                    nc.sync.dma_start(x2t[:], src)
                    sq = sqp.tile([128, NSO, HW], bf16, tag="sq")
                    for so in range(NSO):
                        # in-place: x2t slice becomes diff
                        nc.vector.tensor_tensor(
                            x2t[:, so, :], x2t[:, so, :], x1d[:], sub
                        )
                        nc.scalar.activation(sq[:, so, :], x2t[:, so, :], Square)
                        j = NSO * i + so
                        for q in range(NQ):
                            nc.tensor.matmul(
                                pst[q][:, :],
                                mt[:, j, :],
                                sq[:, so, q * QW : (q + 1) * QW],
                                start=(j == 0),
                                stop=False,
                            )

                nc.vector.tensor_tensor(x2l[:], x2l[:], x1d[0:64, :], sub)
                sql = sqp.tile([64, HW], bf16, tag="sql")
                nc.scalar.activation(sql[:], x2l[:], Square)
                for q in range(NQ):
                    nc.tensor.matmul(
                        pst[q][:, :],
                        mt[0:64, NPAIR - 1, :],
                        sql[:, q * QW : (q + 1) * QW],
                        start=False,
                        stop=True,
                    )

                ot = outp.tile([S, HW], f32, tag="ot")
                for q in range(NQ):
                    nc.scalar.activation(ot[:, q * QW : (q + 1) * QW], pst[q][:], Sqrt)
                # store on the SWDGE ring: a store queued on the sync HWDGE
                # ring would wait on the sqrts and stall every load behind it
                nc.gpsimd.dma_start(out[b].rearrange("(s p) -> s p", s=S), ot[:])

    nc.finalize()
    return nc


def get_nc():
    if "nc" not in _cache:
        _cache["nc"] = _build_nc()
    return _cache["nc"]


def make_mask() -> np.ndarray:
    # mask[j, k, m] = 1 iff partition k of pair-tile j feeds output support m.
    # Pair j < 12 covers supports (2j, 2j+1): k < 64 -> 2j, k >= 64 -> 2j+1.
    # Pair 12 is the leftover single support 24 on partitions 0..63.
    import ml_dtypes

    mask = np.zeros((NPAIR, 128, S), dtype=ml_dtypes.bfloat16)
    for j in range(NPAIR - 1):
        mask[j, 0:64, 2 * j] = 1.0
        mask[j, 64:128, 2 * j + 1] = 1.0
    mask[NPAIR - 1, 0:64, S - 1] = 1.0
    return mask


def make_in_maps(x1: np.ndarray, x2: np.ndarray) -> list[dict]:
    x1 = np.ascontiguousarray(np.asarray(x1, dtype=np.float32)).reshape(B, C, HW)
    x2 = np.ascontiguousarray(np.asarray(x2, dtype=np.float32)).reshape(B, S, C, HW)
    mask = make_mask()
    maps = []
    for i in range(NCORES):
        sl = slice(i * BL, (i + 1) * BL)
        maps.append({"x1": x1[sl], "x2": x2[sl], "mask": mask})
    return maps


def gather_out(results: list[dict]) -> np.ndarray:
    return np.concatenate([np.asarray(r["out"]) for r in results], axis=0).astype(
        np.float32, copy=False
    )


def kernel(x1, x2) -> np.ndarray:
    from concourse.bass_utils import run_bass_kernel_spmd

    nc = get_nc()
    in_maps = make_in_maps(x1, x2)
    res = run_bass_kernel_spmd(nc, in_maps, list(range(NCORES)))
    return gather_out(res.results)
